# revision 1
# baseline (speedup 1.0000x reference)
"""Causal self-attention TRN2 kernel: build + host glue. (v4)

Sharding: tensor-parallel over heads. 16 heads / 8 cores = 2 heads per core.
Each core computes q/k/v for its 2 heads over all 4x2048 tokens, runs causal
attention, and produces a partial output projection outT [1024, 8192]
(wp rows for its heads only). Host sums the 8 partials and transposes.

All matmul operands are fp16 (1 cycle/row, FWL weight loads, fp32 PSUM
accumulation). Softmax math stays fp32 in PSUM.

v4 structure:
- Scores for BOTH heads of one key tile go into one [128, 1024] psum tile
  via two row-tiled matmuls (head A uses PE rows 0-63, head B rows 64-127,
  concurrent), so one ACT exp covers both heads.
- The AV stationary is v_aug [128, 128]: 64 v dims + 64 ones columns, so
  yt rows 64..127 hold the softmax denominator replicated 64x; the
  normalize is then a single partition-shifted DVE divide (no reciprocal,
  no broadcast).
- PE is in-order: AV matmuls are emitted one key-tile behind the scores/exp
  of the current key tile; the output projection of qtile j is deferred
  into the next qtile's instruction stream.
- Rope: one ACT copy stages the qkv psum to SBUF; the swap32 copies and
  mul/mul/add then run on DVE at SBUF speeds.
"""

from contextlib import ExitStack

import numpy as np

import concourse.bacc as bacc
import concourse.bass as bass
import concourse.mybir as mybir
import concourse.tile as tile

F32 = mybir.dt.float32
FP16 = mybir.dt.float16
AF = mybir.ActivationFunctionType
DIV = mybir.AluOpType.divide

D = 1024
H = 16
DH = 64
S = 2048
B = 4
NCORE = 8
HPC = 2  # heads per core
NT = S // 512  # 4 token tiles per batch
NKT = S // 128  # 16 key tiles per batch


def build(nb=B):
    nc = bacc.Bacc("TRN2")
    xT = nc.dram_tensor("xT", [D, B * S], FP16, kind="ExternalInput")
    wq = nc.dram_tensor("wq", [D, 128], FP16, kind="ExternalInput")
    wk = nc.dram_tensor("wk", [D, 128], FP16, kind="ExternalInput")
    wv = nc.dram_tensor("wv", [D, 128], FP16, kind="ExternalInput")
    wp = nc.dram_tensor("wp", [128, D], FP16, kind="ExternalInput")
    cs1 = nc.dram_tensor("cs1", [128, S], F32, kind="ExternalInput")
    cs2 = nc.dram_tensor("cs2", [128, S], F32, kind="ExternalInput")
    masks = nc.dram_tensor("masks", [4, 128, 512], FP16, kind="ExternalInput")
    ident = nc.dram_tensor("ident", [128, 128], FP16, kind="ExternalInput")
    ones1 = nc.dram_tensor("ones1", [128, 64], FP16, kind="ExternalInput")
    outT = nc.dram_tensor("outT", [D, B * S], F32, kind="ExternalOutput")

    with tile.TileContext(nc) as tc, ExitStack() as ctx, nc.allow_low_precision(
        reason="fp16 matmul operands with fp32 accumulation; adequate accuracy"
    ):
        const = ctx.enter_context(tc.tile_pool(name="const", bufs=1))
        xt_pool = ctx.enter_context(tc.tile_pool(name="xt", bufs=4))
        slab = ctx.enter_context(tc.tile_pool(name="slab", bufs=3))
        tmp_pool = ctx.enter_context(tc.tile_pool(name="tmp", bufs=3))
        ex_pool = ctx.enter_context(tc.tile_pool(name="ex", bufs=4))
        ytn_pool = ctx.enter_context(tc.tile_pool(name="ytn", bufs=3))
        ob_pool = ctx.enter_context(tc.tile_pool(name="ob", bufs=4))
        # PSUM budget (8 banks): sc 2x[128,1024]=4, qkv/proj/transpose 2x=2,
        # ytA/ytB 2x=2
        ps_sc = ctx.enter_context(tc.tile_pool(name="ps_sc", bufs=2, space="PSUM"))
        ps_qp = ctx.enter_context(tc.tile_pool(name="ps_qp", bufs=2, space="PSUM"))
        ps_yt = ctx.enter_context(tc.tile_pool(name="ps_yt", bufs=2, space="PSUM"))

        # ---- constants ----
        wq_sb = const.tile([128, 8, 128], FP16)
        wk_sb = const.tile([128, 8, 128], FP16)
        wv_sb = const.tile([128, 8, 128], FP16)
        for dt in range(8):
            nc.sync.dma_start(out=wq_sb[:, dt, :], in_=wq[bass.ts(dt, 128), :])
            nc.sync.dma_start(out=wk_sb[:, dt, :], in_=wk[bass.ts(dt, 128), :])
            nc.sync.dma_start(out=wv_sb[:, dt, :], in_=wv[bass.ts(dt, 128), :])
        wp_sb = const.tile([128, D], FP16)
        nc.sync.dma_start(out=wp_sb[:], in_=wp[:])
        cs1_sb = const.tile([128, S], F32)
        nc.sync.dma_start(out=cs1_sb[:], in_=cs1[:])
        cs2_sb = const.tile([128, S], F32)
        nc.sync.dma_start(out=cs2_sb[:], in_=cs2[:])
        mask_sb = const.tile([128, 4, 512], FP16)
        for d in range(4):
            nc.sync.dma_start(out=mask_sb[:, d, :], in_=masks[d])
        id_sb = const.tile([128, 128], FP16)
        nc.sync.dma_start(out=id_sb[:], in_=ident[:])
        ones_sb = const.tile([128, 64], FP16)
        nc.sync.dma_start(out=ones_sb[:], in_=ones1[:])

        def rope(dst_slice, src_ps, t, nm):
            """dst(fp16) = src*cs1_t + swap32(src)*cs2_t ; src is [128,512] psum."""
            cs1_t = cs1_sb[:, bass.ts(t, 512)]
            cs2_t = cs2_sb[:, bass.ts(t, 512)]
            qs = tmp_pool.tile([128, 512], F32, tag="qs", name="qs")
            nc.scalar.copy(qs[:], src_ps[:])
            sw = tmp_pool.tile([128, 512], F32, tag="sw", name="sw")
            for blk in range(4):
                src_blk = blk + (1 if blk % 2 == 0 else -1)
                nc.vector.tensor_copy(
                    sw[bass.ts(blk, 32), :], qs[bass.ts(src_blk, 32), :]
                )
            t1 = tmp_pool.tile([128, 512], F32, tag="t1", name="t1")
            t2 = tmp_pool.tile([128, 512], F32, tag="t2", name="t2")
            nc.vector.tensor_mul(t1[:], qs[:], cs1_t)
            nc.vector.tensor_mul(t2[:], sw[:], cs2_t)
            nc.vector.tensor_add(dst_slice, t1[:], t2[:])

        pending = []  # deferred emission closures (proj of previous qtile)

        def flush_pending():
            while pending:
                pending.pop(0)()

        for b in range(nb):
            tok0 = b * S
            qT = slab.tile([128, S], FP16, tag="qT", name="qT")
            kT = slab.tile([128, S], FP16, tag="kT", name="kT")
            # per (head, key tile): [128, 128] = [v dims (64) | ones (64)]
            v_sb = slab.tile([128, HPC * NKT * 128], FP16, tag="v_sb", name="v_sb")
            # pre-fill the ones half of every v tile: [128, tile, 64]
            v3d = v_sb[:].rearrange("p (n c) -> p n c", c=128)
            nc.vector.tensor_copy(
                v3d[:, 0 : HPC * NKT, 64:128],
                ones_sb[:, None, :].broadcast_to([128, HPC * NKT, 64]),
            )

            # ---- phase 1 + attention staircase: qtile j only needs
            # key/token tiles 0..j, so attention(j) follows phase1(t=j) ----
            def phase1(t):
                xt8 = xt_pool.tile([128, 8, 512], FP16, tag="xt", name="xt8b")
                for dt in range(8):
                    nc.sync.dma_start(
                        out=xt8[:, dt, :],
                        in_=xT[bass.ts(dt, 128), tok0 + t * 512 : tok0 + (t + 1) * 512],
                    )
                xts = [xt8[:, dt, :] for dt in range(8)]
                q_ps = ps_qp.tile([128, 512], F32, tag="qp", name="q_ps")
                for dt in range(8):
                    nc.tensor.matmul(q_ps[:], wq_sb[:, dt, :], xts[dt][:],
                                     start=dt == 0, stop=dt == 7)
                flush_pending()
                k_ps = ps_qp.tile([128, 512], F32, tag="qp", name="k_ps")
                for dt in range(8):
                    nc.tensor.matmul(k_ps[:], wk_sb[:, dt, :], xts[dt][:],
                                     start=dt == 0, stop=dt == 7)
                rope(qT[:, bass.ts(t, 512)], q_ps, t, f"q{b}_{t}")
                v_ps = ps_qp.tile([128, 512], F32, tag="qp", name="v_ps")
                for dt in range(8):
                    nc.tensor.matmul(v_ps[:], wv_sb[:, dt, :], xts[dt][:],
                                     start=dt == 0, stop=dt == 7)
                rope(kT[:, bass.ts(t, 512)], k_ps, t, f"k{b}_{t}")

                vstage = tmp_pool.tile([128, 512], FP16, tag="vst", name="vstage")
                _phase1_tail(t, vstage, v_ps)

            def _phase1_tail(t, vstage, v_ps):
                nc.scalar.copy(vstage[:], v_ps[:])
                for h in range(HPC):
                    # 4 transposes into one psum tile, one copy out
                    tp4 = ps_qp.tile([128, 256], FP16, tag="qp", name="tp4")
                    for kk in range(4):
                        nc.tensor.transpose(
                            tp4[:, bass.ts(kk, 64)],
                            vstage[bass.ts(h, 64), bass.ts(kk, 128)],
                            id_sb[bass.ts(h, 64), bass.ts(h, 64)],
                        )
                    dst = v_sb[:].rearrange("p (n c) -> p n c", c=128)[
                        :, h * NKT + t * 4 : h * NKT + t * 4 + 4, 0:64
                    ]
                    nc.vector.tensor_copy(dst, tp4[:].rearrange("p (n c) -> p n c", c=64))

            def attention(j, qT=qT, kT=kT, v_sb=v_sb, tok0=tok0):
                yTn = ytn_pool.tile([128, 512], FP16, tag="ytn", name="yTn")
                nkt = 4 * (j + 1)
                yts = {}
                for h in range(HPC):
                    yts[h] = ps_yt.tile([128, 512], F32, tag="yt", name=f"yt{h}")
                exs = {}

                def av_kt(kt, last, j=j, yts=yts, exs=exs):
                    for h in range(HPC):
                        col = (h * NKT + kt) * 128
                        nc.tensor.matmul(
                            yts[h][:],
                            v_sb[:, col : col + 128],
                            exs[kt][:, bass.ts(h, 512)],
                            start=(kt == 0),
                            stop=last,
                        )

                for kt in range(nkt):
                    sc = ps_sc.tile([128, 1024], F32, tag="sc", name="sc")
                    for h in range(HPC):
                        # head A: PE rows 0-63, head B: rows 64-127 (row-tiled,
                        # concurrent); both write their own half of sc
                        nc.tensor.matmul(
                            sc[:, bass.ts(h, 512)],
                            kT[bass.ts(h, 64), bass.ts(kt, 128)],
                            qT[bass.ts(h, 64), bass.ts(j, 512)],
                            start=True,
                            stop=True,
                        )
                    ex = ex_pool.tile([128, 1024], FP16, tag="ex", name="ex")
                    nc.scalar.activation(ex[:], sc[:], AF.Exp, scale=0.125)
                    d = kt - 4 * j
                    if d >= 0:
                        ncols = 128 * (d + 1)
                        for h in range(HPC):
                            nc.vector.tensor_mul(
                                ex[:, h * 512 : h * 512 + ncols],
                                ex[:, h * 512 : h * 512 + ncols],
                                mask_sb[:, d, 0:ncols],
                            )
                    exs[kt] = ex
                    if kt > 1:
                        av_kt(kt - 2, last=False)
                av_kt(nkt - 2, last=False)
                av_kt(nkt - 1, last=True)

                for h in range(HPC):
                    # yt rows 64..127 hold the denominator (ones columns of
                    # v_aug); only one DVE input may come from PSUM, so stage
                    # the denominator rows through SBUF on the scalar engine
                    den = tmp_pool.tile([64, 512], F32, tag="den", name="den")
                    nc.scalar.copy(den[:], yts[h][64:128, :])
                    rc64 = tmp_pool.tile([64, 512], F32, tag="rc64", name="rc64")
                    nc.vector.reciprocal_approx_fast(rc64[:], den[:])
                    nc.vector.tensor_mul(
                        yTn[bass.ts(h, 64), :], yts[h][0:64, :], rc64[:]
                    )

                def proj(j=j, yTn=yTn, tok0=tok0):
                    for dt in range(8):
                        po = ps_qp.tile([128, 512], F32, tag="qp", name="po")
                        nc.tensor.matmul(
                            po[:], wp_sb[:, bass.ts(dt, 128)], yTn[:],
                            start=True, stop=True,
                        )
                        ob = ob_pool.tile([128, 512], F32, tag="ob", name="ob")
                        nc.vector.tensor_copy(ob[:], po[:])
                        nc.sync.dma_start(
                            out=outT[
                                bass.ts(dt, 128), tok0 + j * 512 : tok0 + (j + 1) * 512
                            ],
                            in_=ob[:],
                        )

                pending.append(proj)

            for t in range(NT):
                phase1(t)
                if t > 0:
                    attention(t - 1)
            pending.append(lambda att=attention: att(NT - 1))
        flush_pending()
    nc.finalize()
    return nc


# ---------------- host side ----------------

def host_prepare(x, W_qkv, W_proj):
    xf = np.ascontiguousarray(np.asarray(x, dtype=np.float32).reshape(B * S, D))
    xT = np.ascontiguousarray(xf.T.astype(np.float16))
    Wq = np.asarray(W_qkv[:, 0:D], dtype=np.float32)
    Wk = np.asarray(W_qkv[:, D : 2 * D], dtype=np.float32)
    Wv = np.asarray(W_qkv[:, 2 * D : 3 * D], dtype=np.float32)
    Wp = np.asarray(W_proj, dtype=np.float32)
    perm = np.concatenate([np.arange(0, DH, 2), np.arange(1, DH, 2)])
    half = DH // 2
    inv_freq = 1.0 / (10000.0 ** (np.arange(half, dtype=np.float64) / half))
    freqs = np.outer(np.arange(S, dtype=np.float64), inv_freq)
    cosT = np.cos(freqs).T.astype(np.float32)
    sinT = np.sin(freqs).T.astype(np.float32)
    cs1 = np.concatenate([cosT, cosT, cosT, cosT], axis=0)
    cs2 = np.concatenate([-sinT, sinT, -sinT, sinT], axis=0)
    masks = np.zeros((4, 128, 512), dtype=np.float16)
    for d in range(4):
        ii = np.arange(128)[:, None] + 128 * d
        qq = np.arange(512)[None, :]
        masks[d] = (ii <= qq).astype(np.float16)
    ident = np.eye(128, dtype=np.float16)
    in_maps = []
    for c in range(NCORE):
        hA, hB = HPC * c, HPC * c + 1

        def cols(W, h, p=None):
            w = W[:, h * DH : (h + 1) * DH]
            return w[:, p] if p is not None else w

        in_maps.append(
            {
                "xT": xT,
                "wq": np.ascontiguousarray(
                    np.concatenate([cols(Wq, hA, perm), cols(Wq, hB, perm)], axis=1)
                ).astype(np.float16),
                "wk": np.ascontiguousarray(
                    np.concatenate([cols(Wk, hA, perm), cols(Wk, hB, perm)], axis=1)
                ).astype(np.float16),
                "wv": np.ascontiguousarray(
                    np.concatenate([cols(Wv, hA), cols(Wv, hB)], axis=1)
                ).astype(np.float16),
                "wp": np.ascontiguousarray(Wp[hA * DH : (hB + 1) * DH, :]).astype(
                    np.float16
                ),
                "cs1": cs1,
                "cs2": cs2,
                "masks": masks,
                "ident": ident,
                "ones1": np.ones((128, 64), dtype=np.float16),
            }
        )
    return in_maps




def kernel(x, W_qkv, W_proj):
    """Grading entrypoint: full inputs in, full output out.

    x [4, 2048, 1024] fp32, W_qkv [1024, 3072] fp32, W_proj [1024, 1024] fp32
    -> [4, 2048, 1024] fp32
    """
    from concourse.bass_utils import run_bass_kernel_spmd

    x = np.asarray(x)
    in_maps = host_prepare(x, np.asarray(W_qkv), np.asarray(W_proj))
    nc = build()
    res = run_bass_kernel_spmd(nc, in_maps, list(range(NCORE)))
    acc = np.zeros((D, B * S), dtype=np.float64)
    for c in range(NCORE):
        acc += res.results[c]["outT"].astype(np.float64)
    return np.ascontiguousarray(acc.T.astype(np.float32)).reshape(B, S, D)


def kernel_traced(x, W_qkv, W_proj, trace=False):
    """Dev helper: also returns the BassKernelResults (exec_time_ns etc.)."""
    from concourse.bass_utils import run_bass_kernel_spmd

    in_maps = host_prepare(np.asarray(x), np.asarray(W_qkv), np.asarray(W_proj))
    nc = build()
    res = run_bass_kernel_spmd(nc, in_maps, list(range(NCORE)), trace=trace)
    acc = np.zeros((D, B * S), dtype=np.float64)
    for c in range(NCORE):
        acc += res.results[c]["outT"].astype(np.float64)
    out = np.ascontiguousarray(acc.T.astype(np.float32)).reshape(B, S, D)
    return out, res



# revision 17
# speedup vs baseline: 1.2201x; 1.2201x over previous
"""Causal self-attention TRN2 kernel: build + host glue. (v5)

Sharding: tensor-parallel over heads. 16 heads / 8 cores = 2 heads per core.
Each core computes q/k/v for its 2 heads over all 4x2048 tokens, runs causal
attention, and produces a partial output projection outT [1024, 8192] (fp16)
(wp rows for its heads only). Host sums the 8 partials and transposes.

v5 changes over v4 (402us):
- Software-pipelined emission: the per-token-tile QKV matmuls (and the
  deferred output projection) are interleaved INTO the attention key-tile
  loop as "filler" PE work, so the in-order PE queue never head-of-line
  blocks on the ~1us EXP of each key tile.
- Causal trimming at 128-key granularity: for diagonal key tiles only the
  valid query range [128d, 512) is computed by scores/EXP/AV, and the mask
  multiply shrinks to a single [128,2,128] triangle op.
- Rope in fp16 on DVE with a single stream_shuffle for the pair swap. The
  host permutes rope pairs quadrant-locally (16 even dims | 16 odd dims per
  32-partition quadrant) so the swap is shuffle(mask=(i+16)%32).
- Softmax normalize reads the denominator rows straight from PSUM
  (reciprocal then one multiply per head; no staging copy).
- outT in fp16 (halves output DMA); host accumulates in fp32.
- Startup: first x-tile DMA + wq are issued first; constants after.
"""

from collections import deque
from contextlib import ExitStack

import numpy as np

import concourse.bacc as bacc
import concourse.bass as bass
import concourse.mybir as mybir
import concourse.tile as tile

F32 = mybir.dt.float32
FP16 = mybir.dt.float16
AF = mybir.ActivationFunctionType

D = 1024
H = 16
DH = 64
S = 2048
B = 4
NCORE = 8
HPC = 2  # heads per core
NT = S // 512  # 4 token tiles per batch
NKT = S // 128  # 16 key tiles per batch

SWAP_MASK = [(i + 16) % 32 for i in range(32)]
USE_SHUFFLE = True
USE_AP3D = True
DEBUG = False


def build(nb=B):
    nc = bacc.Bacc("TRN2")
    xT = nc.dram_tensor("xT", [D, B * S], FP16, kind="ExternalInput")
    wq = nc.dram_tensor("wq", [D, 128], FP16, kind="ExternalInput")
    wk = nc.dram_tensor("wk", [D, 128], FP16, kind="ExternalInput")
    wv = nc.dram_tensor("wv", [D, 128], FP16, kind="ExternalInput")
    wp = nc.dram_tensor("wp", [128, D], FP16, kind="ExternalInput")
    cs1 = nc.dram_tensor("cs1", [128, S], FP16, kind="ExternalInput")
    cs2 = nc.dram_tensor("cs2", [128, S], FP16, kind="ExternalInput")
    tri = nc.dram_tensor("tri", [128, 128], FP16, kind="ExternalInput")
    ident = nc.dram_tensor("ident", [128, 128], FP16, kind="ExternalInput")
    ones1 = nc.dram_tensor("ones1", [128, 64], FP16, kind="ExternalInput")
    outT = nc.dram_tensor("outT", [D, B * S], FP16, kind="ExternalOutput")
    if DEBUG:
        dbg_qs = nc.dram_tensor("dbg_qs", [128, 512], FP16, kind="ExternalOutput")
        dbg_qT = nc.dram_tensor("dbg_qT", [128, S], FP16, kind="ExternalOutput")
        dbg_kT = nc.dram_tensor("dbg_kT", [128, S], FP16, kind="ExternalOutput")
        dbg_ex = nc.dram_tensor("dbg_ex", [128, 1024], FP16, kind="ExternalOutput")
        dbg_ytn = nc.dram_tensor("dbg_ytn", [128, 512], FP16, kind="ExternalOutput")
        dbg_vsb = nc.dram_tensor(
            "dbg_vsb", [128, HPC * NKT * 128], FP16, kind="ExternalOutput"
        )
        dbg_yts = nc.dram_tensor("dbg_yts", [128, 1024], F32, kind="ExternalOutput")

    with tile.TileContext(nc) as tc, ExitStack() as ctx, nc.allow_low_precision(
        reason="fp16 matmul operands with fp32 accumulation; adequate accuracy"
    ):
        const = ctx.enter_context(tc.tile_pool(name="const", bufs=1))
        xt_pool = ctx.enter_context(tc.tile_pool(name="xt", bufs=3))
        slab = ctx.enter_context(tc.tile_pool(name="slab", bufs=3))
        tmp_pool = ctx.enter_context(tc.tile_pool(name="tmp", bufs=3))
        ex_pool = ctx.enter_context(tc.tile_pool(name="ex", bufs=4))
        ytn_pool = ctx.enter_context(tc.tile_pool(name="ytn", bufs=3))
        ob_pool = ctx.enter_context(tc.tile_pool(name="ob", bufs=4))
        # PSUM (8 banks): sc 2x[128,1024]fp32=4, yt 2x[128,512]=2, qp 2x=2
        ps_sc = ctx.enter_context(tc.tile_pool(name="ps_sc", bufs=2, space="PSUM"))
        ps_qp = ctx.enter_context(tc.tile_pool(name="ps_qp", bufs=2, space="PSUM"))
        ps_yt = ctx.enter_context(tc.tile_pool(name="ps_yt", bufs=2, space="PSUM"))

        # ---- constants (ordered so the first tile's work can start asap) ----
        wq_sb = const.tile([128, 8, 128], FP16)
        nc.sync.dma_start(
            out=wq_sb[:, 0:4, :], in_=wq[:].rearrange("(a p) c -> p a c", p=128)[:, 0:4]
        )
        nc.sync.dma_start(
            out=wq_sb[:, 4:8, :], in_=wq[:].rearrange("(a p) c -> p a c", p=128)[:, 4:8]
        )
        xt0 = xt_pool.tile([128, 8, 512], FP16, tag="xt", name="xt0")
        for dt in range(8):
            nc.sync.dma_start(out=xt0[:, dt, :], in_=xT[bass.ts(dt, 128), 0:512])
        wk_sb = const.tile([128, 8, 128], FP16)
        wv_sb = const.tile([128, 8, 128], FP16)
        for w_sb, w_dram in ((wk_sb, wk), (wv_sb, wv)):
            nc.sync.dma_start(
                out=w_sb[:, 0:4, :],
                in_=w_dram[:].rearrange("(a p) c -> p a c", p=128)[:, 0:4],
            )
            nc.sync.dma_start(
                out=w_sb[:, 4:8, :],
                in_=w_dram[:].rearrange("(a p) c -> p a c", p=128)[:, 4:8],
            )
        cs1_sb = const.tile([128, S], FP16)
        cs2_sb = const.tile([128, S], FP16)
        nc.sync.dma_start(out=cs1_sb[:, 0:1024], in_=cs1[:, 0:1024])
        nc.sync.dma_start(out=cs2_sb[:, 0:1024], in_=cs2[:, 0:1024])
        id_sb = const.tile([128, 128], FP16)
        nc.sync.dma_start(out=id_sb[:], in_=ident[:])
        ones_sb = const.tile([128, 64], FP16)
        nc.sync.dma_start(out=ones_sb[:], in_=ones1[:])
        tri_sb = const.tile([128, 128], FP16)
        nc.sync.dma_start(out=tri_sb[:], in_=tri[:])
        wp_sb = const.tile([128, D], FP16)
        nc.sync.dma_start(out=wp_sb[:], in_=wp[:])
        nc.sync.dma_start(out=cs1_sb[:, 1024:2048], in_=cs1[:, 1024:2048])
        nc.sync.dma_start(out=cs2_sb[:, 1024:2048], in_=cs2[:, 1024:2048])

        def rope16(dst_slice, qs, t):
            """dst(fp16) = qs*cs1_t + shuffle(qs)*cs2_t ; qs is [128,512] fp16."""
            cs1_t = cs1_sb[:, bass.ts(t, 512)]
            cs2_t = cs2_sb[:, bass.ts(t, 512)]
            sw = tmp_pool.tile([128, 512], FP16, tag="sw", name="sw")
            if USE_SHUFFLE:
                nc.vector.stream_shuffle(sw[:], qs[:], mask=SWAP_MASK)
            else:
                for blk in range(4):
                    src_blk = blk + (1 if blk % 2 == 0 else -1)
                    nc.vector.tensor_copy(
                        sw[bass.ts(blk, 32), :], qs[bass.ts(src_blk, 32), :]
                    )
            t1 = tmp_pool.tile([128, 512], FP16, tag="t1", name="t1")
            nc.vector.tensor_mul(t1[:], qs[:], cs1_t)
            t2 = tmp_pool.tile([128, 512], FP16, tag="t2", name="t2")
            nc.vector.tensor_mul(t2[:], sw[:], cs2_t)
            nc.vector.tensor_add(dst_slice, t1[:], t2[:])

        # batch-generation state (slab tiles rotate per batch)
        cur = {}

        def p1_chunks(b, t, prefetch):
            """Return list of emission closures for token tile t of batch b.

            prefetch: (b', t') of the NEXT token tile, or None; its x DMA is
            emitted inside the first chunk.
            """
            tok0 = b * S

            def c_start():
                if t == 0:
                    qT = slab.tile([128, S], FP16, tag="qT", name="qT")
                    kT = slab.tile([128, S], FP16, tag="kT", name="kT")
                    v_sb = slab.tile(
                        [128, HPC * NKT * 128], FP16, tag="v_sb", name="v_sb"
                    )
                    cur[b] = (qT, kT, v_sb)
                    v3d = v_sb[:].rearrange("p (n c) -> p n c", c=128)
                    nc.vector.tensor_copy(
                        v3d[:, 0 : HPC * NKT, 64:128],
                        ones_sb[:, None, :].broadcast_to([128, HPC * NKT, 64]),
                    )
                if prefetch is not None:
                    pb, pt = prefetch
                    xt_n = xt_pool.tile([128, 8, 512], FP16, tag="xt", name="xt_n")
                    for dt in range(8):
                        nc.sync.dma_start(
                            out=xt_n[:, dt, :],
                            in_=xT[
                                bass.ts(dt, 128),
                                pb * S + pt * 512 : pb * S + (pt + 1) * 512,
                            ],
                        )
                    cur["xt", pb, pt] = xt_n

            xt8 = None
            state = {}

            def get_xt():
                return cur.pop(("xt", b, t)) if ("xt", b, t) in cur else xt0

            def c_q1():
                x = state.setdefault("xt", get_xt())
                q_ps = ps_qp.tile([128, 512], F32, tag="qp", name="q_ps")
                state["q_ps"] = q_ps
                for dt in range(4):
                    nc.tensor.matmul(q_ps[:], wq_sb[:, dt, :], x[:, dt, :],
                                     start=dt == 0, stop=False)

            def c_q2():
                x, q_ps = state["xt"], state["q_ps"]
                for dt in range(4, 8):
                    nc.tensor.matmul(q_ps[:], wq_sb[:, dt, :], x[:, dt, :],
                                     start=False, stop=dt == 7)
                qs = tmp_pool.tile([128, 512], FP16, tag="qs", name="qs")
                nc.scalar.copy(qs[:], q_ps[:])
                state["qs"] = qs
                if DEBUG and b == 0 and t == 0:
                    nc.sync.dma_start(out=dbg_qs[:], in_=qs[:])

            def c_k1():
                x = state["xt"]
                k_ps = ps_qp.tile([128, 512], F32, tag="qp", name="k_ps")
                state["k_ps"] = k_ps
                for dt in range(4):
                    nc.tensor.matmul(k_ps[:], wk_sb[:, dt, :], x[:, dt, :],
                                     start=dt == 0, stop=False)

            def c_k2():
                x, k_ps = state["xt"], state["k_ps"]
                for dt in range(4, 8):
                    nc.tensor.matmul(k_ps[:], wk_sb[:, dt, :], x[:, dt, :],
                                     start=False, stop=dt == 7)
                ks = tmp_pool.tile([128, 512], FP16, tag="ks", name="ks")
                nc.scalar.copy(ks[:], k_ps[:])
                state["ks"] = ks
                rope16(cur[b][0][:, bass.ts(t, 512)], state["qs"], t)

            def c_v1():
                x = state["xt"]
                v_ps = ps_qp.tile([128, 512], F32, tag="qp", name="v_ps")
                state["v_ps"] = v_ps
                for dt in range(4):
                    nc.tensor.matmul(v_ps[:], wv_sb[:, dt, :], x[:, dt, :],
                                     start=dt == 0, stop=False)

            def c_v2():
                x, v_ps = state["xt"], state["v_ps"]
                for dt in range(4, 8):
                    nc.tensor.matmul(v_ps[:], wv_sb[:, dt, :], x[:, dt, :],
                                     start=False, stop=dt == 7)
                vstage = tmp_pool.tile([128, 512], FP16, tag="vst", name="vstage")
                nc.scalar.copy(vstage[:], v_ps[:])
                state["vst"] = vstage
                rope16(cur[b][1][:, bass.ts(t, 512)], state["ks"], t)

            def c_tp(h):
                def run():
                    vstage = state["vst"]
                    tp4 = ps_qp.tile([128, 256], FP16, tag="qp", name="tp4")
                    for kk in range(4):
                        nc.tensor.transpose(
                            tp4[:, bass.ts(kk, 64)],
                            vstage[bass.ts(h, 64), bass.ts(kk, 128)],
                            id_sb[bass.ts(h, 64), bass.ts(h, 64)],
                        )
                    v_sb = cur[b][2]
                    dst = v_sb[:].rearrange("p (n c) -> p n c", c=128)[
                        :, h * NKT + t * 4 : h * NKT + t * 4 + 4, 0:64
                    ]
                    nc.vector.tensor_copy(
                        dst, tp4[:].rearrange("p (n c) -> p n c", c=64)
                    )

                return run

            return [c_start, c_q1, c_q2, c_k1, c_k2, c_v1, c_v2, c_tp(0), c_tp(1)]

        def att_emit(b, j, fill):
            """Emit attention for qtile j of batch b, draining `fill` units
            (independent PE work) evenly across the key-tile slots."""
            qT, kT, v_sb = cur[b]
            tok0 = b * S
            nkt = 4 * (j + 1)
            n_fill = len(fill)
            popped = 0
            yts = {}
            for h in range(HPC):
                yts[h] = ps_yt.tile([128, 512], F32, tag="yt", name=f"yt{h}")
            exs = {}
            starts = {}

            def av_kt(kt, last):
                st = starts[kt]
                ex = exs[kt]
                for h in range(HPC):
                    col = (h * NKT + kt) * 128
                    nc.tensor.matmul(
                        yts[h][:, st:512],
                        v_sb[:, col : col + 128],
                        ex[:, h * 512 + st : (h + 1) * 512],
                        start=(kt == 0),
                        stop=last,
                    )

            for kt in range(nkt):
                d = kt - 4 * j
                st = 128 * max(d, 0)
                starts[kt] = st
                sc = ps_sc.tile([128, 1024], F32, tag="sc", name="sc")
                for h in range(HPC):
                    # head A: PE rows 0-63, head B: rows 64-127 (row-tiled,
                    # concurrent); both write their own half of sc
                    nc.tensor.matmul(
                        sc[:, h * 512 + st : (h + 1) * 512],
                        kT[bass.ts(h, 64), bass.ts(kt, 128)],
                        qT[bass.ts(h, 64), j * 512 + st : (j + 1) * 512],
                        start=True,
                        stop=True,
                    )
                ex = ex_pool.tile([128, 1024], FP16, tag="ex", name="ex")
                if USE_AP3D:
                    sc3 = sc[:].rearrange("p (h q) -> p h q", q=512)
                    ex3 = ex[:].rearrange("p (h q) -> p h q", q=512)
                    nc.scalar.activation(
                        ex3[:, :, st:512], sc3[:, :, st:512], AF.Exp, scale=0.125
                    )
                    if d >= 0:
                        # triangle boundary block [st, st+128) for both heads
                        nc.vector.tensor_mul(
                            ex3[:, :, st : st + 128],
                            ex3[:, :, st : st + 128],
                            tri_sb[:, None, :].broadcast_to([128, 2, 128]),
                        )
                else:
                    for h in range(HPC):
                        nc.scalar.activation(
                            ex[:, h * 512 + st : (h + 1) * 512],
                            sc[:, h * 512 + st : (h + 1) * 512],
                            AF.Exp,
                            scale=0.125,
                        )
                    if d >= 0:
                        for h in range(HPC):
                            nc.vector.tensor_mul(
                                ex[:, h * 512 + st : h * 512 + st + 128],
                                ex[:, h * 512 + st : h * 512 + st + 128],
                                tri_sb[:],
                            )
                if DEBUG and b == 0 and j == 0 and kt == 0:
                    nc.sync.dma_start(out=dbg_ex[:], in_=ex[:])
                    nc.sync.dma_start(out=dbg_qT[:], in_=qT[:])
                    nc.sync.dma_start(out=dbg_kT[:], in_=kT[:])
                exs[kt] = ex
                if kt > 1:
                    av_kt(kt - 2, last=False)
                # drain filler units evenly across slots
                want = (n_fill * (kt + 1) + nkt - 1) // nkt
                while popped < want and fill:
                    fill.popleft()()
                    popped += 1
            av_kt(nkt - 2, last=False)
            av_kt(nkt - 1, last=True)
            while fill and popped < n_fill:
                fill.popleft()()
                popped += 1

            if DEBUG and b == 0 and j == 0:
                nc.sync.dma_start(out=dbg_vsb[:], in_=v_sb[:])
                ysta = tmp_pool.tile([128, 1024], F32, tag="ysta", name="ysta")
                for h in range(HPC):
                    nc.vector.tensor_copy(
                        ysta[:, h * 512 : (h + 1) * 512], yts[h][:]
                    )
                nc.sync.dma_start(out=dbg_yts[:], in_=ysta[:])
            yTn = ytn_pool.tile([128, 512], FP16, tag="ytn", name="yTn")
            for h in range(HPC):
                # yt rows 64..127 hold the denominator (ones columns of
                # v_aug); DVE drops partition offsets on PSUM reads, so the
                # den rows must be staged through SBUF on the scalar engine
                den = tmp_pool.tile([64, 512], F32, tag="den", name="den")
                nc.scalar.copy(den[:], yts[h][64:128, :])
                rc64 = tmp_pool.tile([64, 512], F32, tag="rc64", name="rc64")
                nc.vector.reciprocal_approx_fast(rc64[:], den[:])
                nc.vector.tensor_mul(
                    yTn[bass.ts(h, 64), :], yts[h][0:64, :], rc64[:]
                )

            if DEBUG and b == 0 and j == 0:
                nc.sync.dma_start(out=dbg_ytn[:], in_=yTn[:])

            def proj_chunk(dt):
                def run():
                    po = ps_qp.tile([128, 512], F32, tag="qp", name="po")
                    nc.tensor.matmul(
                        po[:], wp_sb[:, bass.ts(dt, 128)], yTn[:],
                        start=True, stop=True,
                    )
                    ob = ob_pool.tile([128, 512], FP16, tag="ob", name="ob")
                    nc.vector.tensor_copy(ob[:], po[:])
                    nc.sync.dma_start(
                        out=outT[
                            bass.ts(dt, 128), tok0 + j * 512 : tok0 + (j + 1) * 512
                        ],
                        in_=ob[:],
                    )

                return run

            return [proj_chunk(dt) for dt in range(8)]

        # ---- driver: p1 units run 2 steps ahead of att units ----
        p1s = [(b, t) for b in range(nb) for t in range(NT)]
        atts = [(b, j) for b in range(nb) for j in range(NT)]
        fill = deque()
        for i in range(len(p1s) + 2):
            if i < len(p1s):
                pref = p1s[i + 1] if i + 1 < len(p1s) else None
                fill.extend(p1_chunks(*p1s[i], prefetch=pref))
            if i >= 2:
                proj = att_emit(*atts[i - 2], fill=fill)
                fill.extend(proj)
            elif i < 2:
                while fill:
                    fill.popleft()()
        while fill:
            fill.popleft()()
    nc.finalize()
    return nc


# ---------------- host side ----------------

def host_prepare(x, W_qkv, W_proj):
    xf = np.ascontiguousarray(np.asarray(x, dtype=np.float32).reshape(B * S, D))
    xT = np.ascontiguousarray(xf.T.astype(np.float16))
    Wq = np.asarray(W_qkv[:, 0:D], dtype=np.float32)
    Wk = np.asarray(W_qkv[:, D : 2 * D], dtype=np.float32)
    Wv = np.asarray(W_qkv[:, 2 * D : 3 * D], dtype=np.float32)
    Wp = np.asarray(W_proj, dtype=np.float32)
    half = DH // 2
    inv_freq = 1.0 / (10000.0 ** (np.arange(half, dtype=np.float64) / half))
    freqs = np.outer(np.arange(S, dtype=np.float64), inv_freq)  # [S, 32]
    cos = np.cos(freqs)
    sin = np.sin(freqs)
    if USE_SHUFFLE:
        # quadrant-local rope pair layout: per 32-slot quadrant q, slots 0-15
        # hold even dims of pairs 16q..16q+15, slots 16-31 the odd dims.
        perm = np.empty(DH, dtype=np.int64)
        cs1_h = np.empty((DH, S), dtype=np.float32)
        cs2_h = np.empty((DH, S), dtype=np.float32)
        for q in range(2):
            for i in range(32):
                k = 16 * q + (i % 16)
                r = 32 * q + i
                perm[r] = 2 * k if i < 16 else 2 * k + 1
                cs1_h[r] = cos[:, k]
                cs2_h[r] = -sin[:, k] if i < 16 else sin[:, k]
    else:
        # v4 layout: [32 evens | 32 odds] per head; swap = 32-block pair swap
        perm = np.concatenate([np.arange(0, DH, 2), np.arange(1, DH, 2)])
        cosT = cos.T.astype(np.float32)
        sinT = sin.T.astype(np.float32)
        cs1_h = np.concatenate([cosT, cosT], axis=0)
        cs2_h = np.concatenate([-sinT, sinT], axis=0)
    cs1 = np.concatenate([cs1_h, cs1_h], axis=0).astype(np.float16)
    cs2 = np.concatenate([cs2_h, cs2_h], axis=0).astype(np.float16)
    ii = np.arange(128)[:, None]
    qq = np.arange(128)[None, :]
    tri = (ii <= qq).astype(np.float16)
    ident = np.eye(128, dtype=np.float16)
    in_maps = []
    for c in range(NCORE):
        hA, hB = HPC * c, HPC * c + 1

        def cols(W, h, p=None):
            w = W[:, h * DH : (h + 1) * DH]
            return w[:, p] if p is not None else w

        in_maps.append(
            {
                "xT": xT,
                "wq": np.ascontiguousarray(
                    np.concatenate([cols(Wq, hA, perm), cols(Wq, hB, perm)], axis=1)
                ).astype(np.float16),
                "wk": np.ascontiguousarray(
                    np.concatenate([cols(Wk, hA, perm), cols(Wk, hB, perm)], axis=1)
                ).astype(np.float16),
                "wv": np.ascontiguousarray(
                    np.concatenate([cols(Wv, hA), cols(Wv, hB)], axis=1)
                ).astype(np.float16),
                "wp": np.ascontiguousarray(Wp[hA * DH : (hB + 1) * DH, :]).astype(
                    np.float16
                ),
                "cs1": cs1,
                "cs2": cs2,
                "tri": tri,
                "ident": ident,
                "ones1": np.ones((128, 64), dtype=np.float16),
            }
        )
    return in_maps




def kernel(x, W_qkv, W_proj):
    """Grading entrypoint: full inputs in, full output out.

    x [4, 2048, 1024] fp32, W_qkv [1024, 3072] fp32, W_proj [1024, 1024] fp32
    -> [4, 2048, 1024] fp32
    """
    from concourse.bass_utils import run_bass_kernel_spmd

    x = np.asarray(x)
    in_maps = host_prepare(x, np.asarray(W_qkv), np.asarray(W_proj))
    nc = build()
    res = run_bass_kernel_spmd(nc, in_maps, list(range(NCORE)))
    acc = np.zeros((D, B * S), dtype=np.float32)
    for c in range(NCORE):
        acc += res.results[c]["outT"].astype(np.float32)
    return np.ascontiguousarray(acc.T).reshape(B, S, D)


def kernel_traced(x, W_qkv, W_proj, trace=False):
    """Dev helper: also returns the BassKernelResults (exec_time_ns etc.)."""
    from concourse.bass_utils import run_bass_kernel_spmd

    in_maps = host_prepare(np.asarray(x), np.asarray(W_qkv), np.asarray(W_proj))
    nc = build()
    res = run_bass_kernel_spmd(nc, in_maps, list(range(NCORE)), trace=trace)
    acc = np.zeros((D, B * S), dtype=np.float32)
    for c in range(NCORE):
        acc += res.results[c]["outT"].astype(np.float32)
    out = np.ascontiguousarray(acc.T).reshape(B, S, D)
    return out, res


# revision 20
# speedup vs baseline: 1.2535x; 1.0274x over previous
"""Causal self-attention TRN2 kernel: build + host glue. (v5)

Sharding: tensor-parallel over heads. 16 heads / 8 cores = 2 heads per core.
Each core computes q/k/v for its 2 heads over all 4x2048 tokens, runs causal
attention, and produces a partial output projection outT [1024, 8192] (fp16)
(wp rows for its heads only). Host sums the 8 partials and transposes.

v5 changes over v4 (402us):
- Software-pipelined emission: the per-token-tile QKV matmuls (and the
  deferred output projection) are interleaved INTO the attention key-tile
  loop as "filler" PE work, so the in-order PE queue never head-of-line
  blocks on the ~1us EXP of each key tile.
- Causal trimming at 128-key granularity: for diagonal key tiles only the
  valid query range [128d, 512) is computed by scores/EXP/AV, and the mask
  multiply shrinks to a single [128,2,128] triangle op.
- Rope in fp16 on DVE with a single stream_shuffle for the pair swap. The
  host permutes rope pairs quadrant-locally (16 even dims | 16 odd dims per
  32-partition quadrant) so the swap is shuffle(mask=(i+16)%32).
- Softmax normalize reads the denominator rows straight from PSUM
  (reciprocal then one multiply per head; no staging copy).
- outT in fp16 (halves output DMA); host accumulates in fp32.
- Startup: first x-tile DMA + wq are issued first; constants after.
"""

from collections import deque
from contextlib import ExitStack

import numpy as np

import concourse.bacc as bacc
import concourse.bass as bass
import concourse.mybir as mybir
import concourse.tile as tile

F32 = mybir.dt.float32
FP16 = mybir.dt.float16
AF = mybir.ActivationFunctionType

D = 1024
H = 16
DH = 64
S = 2048
B = 4
NCORE = 8
HPC = 2  # heads per core
NT = S // 512  # 4 token tiles per batch
NKT = S // 128  # 16 key tiles per batch

SWAP_MASK = [(i + 16) % 32 for i in range(32)]
USE_SHUFFLE = True
USE_AP3D = True
DEBUG = False


def build(nb=B):
    nc = bacc.Bacc("TRN2")
    xT = nc.dram_tensor("xT", [D, B * S], FP16, kind="ExternalInput")
    wq = nc.dram_tensor("wq", [D, 128], FP16, kind="ExternalInput")
    wk = nc.dram_tensor("wk", [D, 128], FP16, kind="ExternalInput")
    wv = nc.dram_tensor("wv", [D, 128], FP16, kind="ExternalInput")
    wp = nc.dram_tensor("wp", [128, D], FP16, kind="ExternalInput")
    cs1 = nc.dram_tensor("cs1", [128, S], FP16, kind="ExternalInput")
    cs2 = nc.dram_tensor("cs2", [128, S], FP16, kind="ExternalInput")
    tri = nc.dram_tensor("tri", [128, 128], FP16, kind="ExternalInput")
    ident = nc.dram_tensor("ident", [128, 128], FP16, kind="ExternalInput")
    ones1 = nc.dram_tensor("ones1", [128, 64], FP16, kind="ExternalInput")
    outT = nc.dram_tensor("outT", [D, B * S], FP16, kind="ExternalOutput")
    if DEBUG:
        dbg_qs = nc.dram_tensor("dbg_qs", [128, 512], FP16, kind="ExternalOutput")
        dbg_qT = nc.dram_tensor("dbg_qT", [128, S], FP16, kind="ExternalOutput")
        dbg_kT = nc.dram_tensor("dbg_kT", [128, S], FP16, kind="ExternalOutput")
        dbg_ex = nc.dram_tensor("dbg_ex", [128, 1024], FP16, kind="ExternalOutput")
        dbg_ytn = nc.dram_tensor("dbg_ytn", [128, 512], FP16, kind="ExternalOutput")
        dbg_vsb = nc.dram_tensor(
            "dbg_vsb", [128, HPC * NKT * 128], FP16, kind="ExternalOutput"
        )
        dbg_yts = nc.dram_tensor("dbg_yts", [128, 1024], F32, kind="ExternalOutput")

    with tile.TileContext(nc) as tc, ExitStack() as ctx, nc.allow_low_precision(
        reason="fp16 matmul operands with fp32 accumulation; adequate accuracy"
    ):
        const = ctx.enter_context(tc.tile_pool(name="const", bufs=1))
        xt_pool = ctx.enter_context(tc.tile_pool(name="xt", bufs=3))
        slab = ctx.enter_context(tc.tile_pool(name="slab", bufs=3))
        tmp_pool = ctx.enter_context(tc.tile_pool(name="tmp", bufs=3))
        ex_pool = ctx.enter_context(tc.tile_pool(name="ex", bufs=4))
        ytn_pool = ctx.enter_context(tc.tile_pool(name="ytn", bufs=3))
        ob_pool = ctx.enter_context(tc.tile_pool(name="ob", bufs=4))
        # PSUM (8 banks): sc 2x[128,1024]fp32=4, yt 2x[128,512]=2, qp 2x=2
        ps_sc = ctx.enter_context(tc.tile_pool(name="ps_sc", bufs=2, space="PSUM"))
        ps_qp = ctx.enter_context(tc.tile_pool(name="ps_qp", bufs=2, space="PSUM"))
        ps_yt = ctx.enter_context(tc.tile_pool(name="ps_yt", bufs=2, space="PSUM"))

        # ---- constants (ordered so the first tile's work can start asap) ----
        wq_sb = const.tile([128, 8, 128], FP16)
        nc.sync.dma_start(
            out=wq_sb[:, 0:4, :], in_=wq[:].rearrange("(a p) c -> p a c", p=128)[:, 0:4]
        )
        nc.sync.dma_start(
            out=wq_sb[:, 4:8, :], in_=wq[:].rearrange("(a p) c -> p a c", p=128)[:, 4:8]
        )
        xt0 = xt_pool.tile([128, 8, 512], FP16, tag="xt", name="xt0")
        for dt in range(8):
            nc.sync.dma_start(out=xt0[:, dt, :], in_=xT[bass.ts(dt, 128), 0:512])
        wk_sb = const.tile([128, 8, 128], FP16)
        wv_sb = const.tile([128, 8, 128], FP16)
        for w_sb, w_dram in ((wk_sb, wk), (wv_sb, wv)):
            nc.sync.dma_start(
                out=w_sb[:, 0:4, :],
                in_=w_dram[:].rearrange("(a p) c -> p a c", p=128)[:, 0:4],
            )
            nc.sync.dma_start(
                out=w_sb[:, 4:8, :],
                in_=w_dram[:].rearrange("(a p) c -> p a c", p=128)[:, 4:8],
            )
        cs1_sb = const.tile([128, S], FP16)
        cs2_sb = const.tile([128, S], FP16)
        nc.sync.dma_start(out=cs1_sb[:, 0:1024], in_=cs1[:, 0:1024])
        nc.sync.dma_start(out=cs2_sb[:, 0:1024], in_=cs2[:, 0:1024])
        id_sb = const.tile([128, 128], FP16)
        nc.sync.dma_start(out=id_sb[:], in_=ident[:])
        ones_sb = const.tile([128, 64], FP16)
        nc.sync.dma_start(out=ones_sb[:], in_=ones1[:])
        tri_sb = const.tile([128, 128], FP16)
        nc.sync.dma_start(out=tri_sb[:], in_=tri[:])
        wp_sb = const.tile([128, D], FP16)
        nc.sync.dma_start(out=wp_sb[:], in_=wp[:])
        nc.sync.dma_start(out=cs1_sb[:, 1024:2048], in_=cs1[:, 1024:2048])
        nc.sync.dma_start(out=cs2_sb[:, 1024:2048], in_=cs2[:, 1024:2048])

        def rope16(dst_slice, qs, t):
            """dst(fp16) = qs*cs1_t + shuffle(qs)*cs2_t ; qs is [128,512] fp16."""
            cs1_t = cs1_sb[:, bass.ts(t, 512)]
            cs2_t = cs2_sb[:, bass.ts(t, 512)]
            sw = tmp_pool.tile([128, 512], FP16, tag="sw", name="sw")
            if USE_SHUFFLE:
                nc.vector.stream_shuffle(sw[:], qs[:], mask=SWAP_MASK)
            else:
                for blk in range(4):
                    src_blk = blk + (1 if blk % 2 == 0 else -1)
                    nc.vector.tensor_copy(
                        sw[bass.ts(blk, 32), :], qs[bass.ts(src_blk, 32), :]
                    )
            t1 = tmp_pool.tile([128, 512], FP16, tag="t1", name="t1")
            nc.vector.tensor_mul(t1[:], qs[:], cs1_t)
            t2 = tmp_pool.tile([128, 512], FP16, tag="t2", name="t2")
            nc.vector.tensor_mul(t2[:], sw[:], cs2_t)
            nc.vector.tensor_add(dst_slice, t1[:], t2[:])

        # batch-generation state (slab tiles rotate per batch)
        cur = {}

        def p1_chunks(b, t, prefetch):
            """Return list of emission closures for token tile t of batch b.

            prefetch: (b', t') of the NEXT token tile, or None; its x DMA is
            emitted inside the first chunk.
            """
            tok0 = b * S

            def c_start():
                if t == 0:
                    qT = slab.tile([128, S], FP16, tag="qT", name="qT")
                    kT = slab.tile([128, S], FP16, tag="kT", name="kT")
                    v_sb = slab.tile(
                        [128, HPC * NKT * 128], FP16, tag="v_sb", name="v_sb"
                    )
                    cur[b] = (qT, kT, v_sb)
                    v3d = v_sb[:].rearrange("p (n c) -> p n c", c=128)
                    nc.vector.tensor_copy(
                        v3d[:, 0 : HPC * NKT, 64:128],
                        ones_sb[:, None, :].broadcast_to([128, HPC * NKT, 64]),
                    )
                if prefetch is not None:
                    pb, pt = prefetch
                    xt_n = xt_pool.tile([128, 8, 512], FP16, tag="xt", name="xt_n")
                    for dt in range(8):
                        nc.sync.dma_start(
                            out=xt_n[:, dt, :],
                            in_=xT[
                                bass.ts(dt, 128),
                                pb * S + pt * 512 : pb * S + (pt + 1) * 512,
                            ],
                        )
                    cur["xt", pb, pt] = xt_n

            xt8 = None
            state = {}

            def get_xt():
                return cur.pop(("xt", b, t)) if ("xt", b, t) in cur else xt0

            def c_q1():
                x = state.setdefault("xt", get_xt())
                q_ps = ps_qp.tile([128, 512], F32, tag="qp", name="q_ps")
                state["q_ps"] = q_ps
                for dt in range(4):
                    nc.tensor.matmul(q_ps[:], wq_sb[:, dt, :], x[:, dt, :],
                                     start=dt == 0, stop=False)

            def c_q2():
                x, q_ps = state["xt"], state["q_ps"]
                for dt in range(4, 8):
                    nc.tensor.matmul(q_ps[:], wq_sb[:, dt, :], x[:, dt, :],
                                     start=False, stop=dt == 7)
                qs = tmp_pool.tile([128, 512], FP16, tag="qs", name="qs")
                nc.scalar.copy(qs[:], q_ps[:])
                state["qs"] = qs
                if DEBUG and b == 0 and t == 0:
                    nc.sync.dma_start(out=dbg_qs[:], in_=qs[:])

            def c_k1():
                x = state["xt"]
                k_ps = ps_qp.tile([128, 512], F32, tag="qp", name="k_ps")
                state["k_ps"] = k_ps
                for dt in range(4):
                    nc.tensor.matmul(k_ps[:], wk_sb[:, dt, :], x[:, dt, :],
                                     start=dt == 0, stop=False)

            def c_k2():
                x, k_ps = state["xt"], state["k_ps"]
                for dt in range(4, 8):
                    nc.tensor.matmul(k_ps[:], wk_sb[:, dt, :], x[:, dt, :],
                                     start=False, stop=dt == 7)
                ks = tmp_pool.tile([128, 512], FP16, tag="ks", name="ks")
                nc.scalar.copy(ks[:], k_ps[:])
                state["ks"] = ks
                rope16(cur[b][0][:, bass.ts(t, 512)], state["qs"], t)

            def c_v1():
                x = state["xt"]
                v_ps = ps_qp.tile([128, 512], F32, tag="qp", name="v_ps")
                state["v_ps"] = v_ps
                for dt in range(4):
                    nc.tensor.matmul(v_ps[:], wv_sb[:, dt, :], x[:, dt, :],
                                     start=dt == 0, stop=False)

            def c_v2():
                x, v_ps = state["xt"], state["v_ps"]
                for dt in range(4, 8):
                    nc.tensor.matmul(v_ps[:], wv_sb[:, dt, :], x[:, dt, :],
                                     start=False, stop=dt == 7)
                vstage = tmp_pool.tile([128, 512], FP16, tag="vst", name="vstage")
                nc.scalar.copy(vstage[:], v_ps[:])
                state["vst"] = vstage
                rope16(cur[b][1][:, bass.ts(t, 512)], state["ks"], t)

            def c_tp(h):
                def run():
                    vstage = state["vst"]
                    tp4 = ps_qp.tile([128, 256], FP16, tag="qp", name="tp4")
                    for kk in range(4):
                        nc.tensor.transpose(
                            tp4[:, bass.ts(kk, 64)],
                            vstage[bass.ts(h, 64), bass.ts(kk, 128)],
                            id_sb[bass.ts(h, 64), bass.ts(h, 64)],
                        )
                    v_sb = cur[b][2]
                    dst = v_sb[:].rearrange("p (n c) -> p n c", c=128)[
                        :, h * NKT + t * 4 : h * NKT + t * 4 + 4, 0:64
                    ]
                    nc.vector.tensor_copy(
                        dst, tp4[:].rearrange("p (n c) -> p n c", c=64)
                    )

                return run

            return [c_start, c_q1, c_q2, c_k1, c_k2, c_v1, c_v2, c_tp(0), c_tp(1)]

        def att_emit(b, j, fill):
            """Emit attention for qtile j of batch b, draining `fill` units
            (independent PE work) evenly across the key-tile slots."""
            qT, kT, v_sb = cur[b]
            tok0 = b * S
            nkt = 4 * (j + 1)
            n_fill = len(fill)
            popped = 0
            yts = {}
            for h in range(HPC):
                yts[h] = ps_yt.tile([128, 512], F32, tag="yt", name=f"yt{h}")
            exs = {}
            starts = {}

            def av_kt(kt, last):
                st = starts[kt]
                ex = exs[kt]
                for h in range(HPC):
                    col = (h * NKT + kt) * 128
                    nc.tensor.matmul(
                        yts[h][:, st:512],
                        v_sb[:, col : col + 128],
                        ex[:, h * 512 + st : (h + 1) * 512],
                        start=(kt == 0),
                        stop=last,
                    )

            for kt in range(nkt):
                d = kt - 4 * j
                st = 128 * max(d, 0)
                starts[kt] = st
                sc = ps_sc.tile([128, 1024], F32, tag="sc", name="sc")
                for h in range(HPC):
                    # head A: PE rows 0-63, head B: rows 64-127 (row-tiled,
                    # concurrent); both write their own half of sc
                    nc.tensor.matmul(
                        sc[:, h * 512 + st : (h + 1) * 512],
                        kT[bass.ts(h, 64), bass.ts(kt, 128)],
                        qT[bass.ts(h, 64), j * 512 + st : (j + 1) * 512],
                        start=True,
                        stop=True,
                    )
                ex = ex_pool.tile([128, 1024], FP16, tag="ex", name="ex")
                if USE_AP3D:
                    sc3 = sc[:].rearrange("p (h q) -> p h q", q=512)
                    ex3 = ex[:].rearrange("p (h q) -> p h q", q=512)
                    nc.scalar.activation(
                        ex3[:, :, st:512], sc3[:, :, st:512], AF.Exp, scale=0.125
                    )
                    if d >= 0:
                        # triangle boundary block [st, st+128) for both heads
                        nc.vector.tensor_mul(
                            ex3[:, :, st : st + 128],
                            ex3[:, :, st : st + 128],
                            tri_sb[:, None, :].broadcast_to([128, 2, 128]),
                        )
                else:
                    for h in range(HPC):
                        nc.scalar.activation(
                            ex[:, h * 512 + st : (h + 1) * 512],
                            sc[:, h * 512 + st : (h + 1) * 512],
                            AF.Exp,
                            scale=0.125,
                        )
                    if d >= 0:
                        for h in range(HPC):
                            nc.vector.tensor_mul(
                                ex[:, h * 512 + st : h * 512 + st + 128],
                                ex[:, h * 512 + st : h * 512 + st + 128],
                                tri_sb[:],
                            )
                if DEBUG and b == 0 and j == 0 and kt == 0:
                    nc.sync.dma_start(out=dbg_ex[:], in_=ex[:])
                    nc.sync.dma_start(out=dbg_qT[:], in_=qT[:])
                    nc.sync.dma_start(out=dbg_kT[:], in_=kT[:])
                exs[kt] = ex
                if kt > 1:
                    av_kt(kt - 2, last=False)
                # drain filler units evenly, holding some back for the att
                # boundary (emitted after the av tail, so PE has
                # EXP-independent work while ACT drains the final EXPs)
                want = (n_fill * (kt + 1)) // (nkt + 2)
                while popped < want and fill:
                    fill.popleft()()
                    popped += 1
            av_kt(nkt - 2, last=False)
            av_kt(nkt - 1, last=True)
            while fill and popped < n_fill:
                fill.popleft()()
                popped += 1

            if DEBUG and b == 0 and j == 0:
                nc.sync.dma_start(out=dbg_vsb[:], in_=v_sb[:])
                ysta = tmp_pool.tile([128, 1024], F32, tag="ysta", name="ysta")
                for h in range(HPC):
                    nc.vector.tensor_copy(
                        ysta[:, h * 512 : (h + 1) * 512], yts[h][:]
                    )
                nc.sync.dma_start(out=dbg_yts[:], in_=ysta[:])
            yTn = ytn_pool.tile([128, 512], FP16, tag="ytn", name="yTn")

            def norm_chunk():
                for h in range(HPC):
                    # yt rows 64..127 hold the denominator (ones columns of
                    # v_aug); DVE drops partition offsets on PSUM reads, so
                    # the den rows are staged through SBUF on the scalar
                    # engine. Deferred into the next att's filler stream so
                    # these ACT copies don't delay its first EXPs.
                    den = tmp_pool.tile([64, 512], F32, tag="den", name="den")
                    nc.scalar.copy(den[:], yts[h][64:128, :])
                    rc64 = tmp_pool.tile([64, 512], F32, tag="rc64", name="rc64")
                    nc.vector.reciprocal_approx_fast(rc64[:], den[:])
                    nc.vector.tensor_mul(
                        yTn[bass.ts(h, 64), :], yts[h][0:64, :], rc64[:]
                    )
                if DEBUG and b == 0 and j == 0:
                    nc.sync.dma_start(out=dbg_ytn[:], in_=yTn[:])

            def proj_chunk(dt):
                def run():
                    po = ps_qp.tile([128, 512], F32, tag="qp", name="po")
                    nc.tensor.matmul(
                        po[:], wp_sb[:, bass.ts(dt, 128)], yTn[:],
                        start=True, stop=True,
                    )
                    ob = ob_pool.tile([128, 512], FP16, tag="ob", name="ob")
                    nc.vector.tensor_copy(ob[:], po[:])
                    nc.sync.dma_start(
                        out=outT[
                            bass.ts(dt, 128), tok0 + j * 512 : tok0 + (j + 1) * 512
                        ],
                        in_=ob[:],
                    )

                return run

            return [norm_chunk] + [proj_chunk(dt) for dt in range(8)]

        # ---- driver: p1 units run 2 steps ahead of att units ----
        p1s = [(b, t) for b in range(nb) for t in range(NT)]
        atts = [(b, j) for b in range(nb) for j in range(NT)]
        fill = deque()
        for i in range(len(p1s) + 2):
            if i < len(p1s):
                pref = p1s[i + 1] if i + 1 < len(p1s) else None
                fill.extend(p1_chunks(*p1s[i], prefetch=pref))
            if i >= 2:
                proj = att_emit(*atts[i - 2], fill=fill)
                fill.extend(proj)
            elif i < 2:
                while fill:
                    fill.popleft()()
        while fill:
            fill.popleft()()
    nc.finalize()
    return nc


# ---------------- host side ----------------

def host_prepare(x, W_qkv, W_proj):
    xf = np.ascontiguousarray(np.asarray(x, dtype=np.float32).reshape(B * S, D))
    xT = np.ascontiguousarray(xf.T.astype(np.float16))
    Wq = np.asarray(W_qkv[:, 0:D], dtype=np.float32)
    Wk = np.asarray(W_qkv[:, D : 2 * D], dtype=np.float32)
    Wv = np.asarray(W_qkv[:, 2 * D : 3 * D], dtype=np.float32)
    Wp = np.asarray(W_proj, dtype=np.float32)
    half = DH // 2
    inv_freq = 1.0 / (10000.0 ** (np.arange(half, dtype=np.float64) / half))
    freqs = np.outer(np.arange(S, dtype=np.float64), inv_freq)  # [S, 32]
    cos = np.cos(freqs)
    sin = np.sin(freqs)
    if USE_SHUFFLE:
        # quadrant-local rope pair layout: per 32-slot quadrant q, slots 0-15
        # hold even dims of pairs 16q..16q+15, slots 16-31 the odd dims.
        perm = np.empty(DH, dtype=np.int64)
        cs1_h = np.empty((DH, S), dtype=np.float32)
        cs2_h = np.empty((DH, S), dtype=np.float32)
        for q in range(2):
            for i in range(32):
                k = 16 * q + (i % 16)
                r = 32 * q + i
                perm[r] = 2 * k if i < 16 else 2 * k + 1
                cs1_h[r] = cos[:, k]
                cs2_h[r] = -sin[:, k] if i < 16 else sin[:, k]
    else:
        # v4 layout: [32 evens | 32 odds] per head; swap = 32-block pair swap
        perm = np.concatenate([np.arange(0, DH, 2), np.arange(1, DH, 2)])
        cosT = cos.T.astype(np.float32)
        sinT = sin.T.astype(np.float32)
        cs1_h = np.concatenate([cosT, cosT], axis=0)
        cs2_h = np.concatenate([-sinT, sinT], axis=0)
    cs1 = np.concatenate([cs1_h, cs1_h], axis=0).astype(np.float16)
    cs2 = np.concatenate([cs2_h, cs2_h], axis=0).astype(np.float16)
    ii = np.arange(128)[:, None]
    qq = np.arange(128)[None, :]
    tri = (ii <= qq).astype(np.float16)
    ident = np.eye(128, dtype=np.float16)
    in_maps = []
    for c in range(NCORE):
        hA, hB = HPC * c, HPC * c + 1

        def cols(W, h, p=None):
            w = W[:, h * DH : (h + 1) * DH]
            return w[:, p] if p is not None else w

        in_maps.append(
            {
                "xT": xT,
                "wq": np.ascontiguousarray(
                    np.concatenate([cols(Wq, hA, perm), cols(Wq, hB, perm)], axis=1)
                ).astype(np.float16),
                "wk": np.ascontiguousarray(
                    np.concatenate([cols(Wk, hA, perm), cols(Wk, hB, perm)], axis=1)
                ).astype(np.float16),
                "wv": np.ascontiguousarray(
                    np.concatenate([cols(Wv, hA), cols(Wv, hB)], axis=1)
                ).astype(np.float16),
                "wp": np.ascontiguousarray(Wp[hA * DH : (hB + 1) * DH, :]).astype(
                    np.float16
                ),
                "cs1": cs1,
                "cs2": cs2,
                "tri": tri,
                "ident": ident,
                "ones1": np.ones((128, 64), dtype=np.float16),
            }
        )
    return in_maps




def kernel(x, W_qkv, W_proj):
    """Grading entrypoint: full inputs in, full output out.

    x [4, 2048, 1024] fp32, W_qkv [1024, 3072] fp32, W_proj [1024, 1024] fp32
    -> [4, 2048, 1024] fp32
    """
    from concourse.bass_utils import run_bass_kernel_spmd

    x = np.asarray(x)
    in_maps = host_prepare(x, np.asarray(W_qkv), np.asarray(W_proj))
    nc = build()
    res = run_bass_kernel_spmd(nc, in_maps, list(range(NCORE)))
    acc = np.zeros((D, B * S), dtype=np.float32)
    for c in range(NCORE):
        acc += res.results[c]["outT"].astype(np.float32)
    return np.ascontiguousarray(acc.T).reshape(B, S, D)


def kernel_traced(x, W_qkv, W_proj, trace=False):
    """Dev helper: also returns the BassKernelResults (exec_time_ns etc.)."""
    from concourse.bass_utils import run_bass_kernel_spmd

    in_maps = host_prepare(np.asarray(x), np.asarray(W_qkv), np.asarray(W_proj))
    nc = build()
    res = run_bass_kernel_spmd(nc, in_maps, list(range(NCORE)), trace=trace)
    acc = np.zeros((D, B * S), dtype=np.float32)
    for c in range(NCORE):
        acc += res.results[c]["outT"].astype(np.float32)
    out = np.ascontiguousarray(acc.T).reshape(B, S, D)
    return out, res
